# revision 1
# baseline (speedup 1.0000x reference)
"""YOLO loss kernel for Trainium2 (Bass/Tile), data-parallel over 8 NeuronCores.

Math (per sample n, cell s; S=14, SS=196, B=2, C=20, D=30):
  t4 = target conf channel (binary 0/1, channels 4 and 9 identical)
  obj = t4, noobj = 1 - t4 (exact, since t4 is binary)
  For box b (pred channels o=5b..o+4):
    corners from (cx/S +- w/2), intersect with target box (channels 0..3),
    inter = relu(ox)*relu(oy), union = t_area + p_area - inter,
    iou_b = inter / (union + (union==0))
  sel = (iou1 > iou0)              # responsible box (argmax, first-on-ties)
  selm = sel*t4 ; s0m = t4 - selm  # binary masks
  coord  = sum 5*[s0m*(p_k-t_k)^2 + selm*(p_{5+k}-t_{5+k})^2], k=0..3
  conf   = sum s0m*(p4-iou0)^2 + selm*(p9-iou1)^2
  noobj  = sum 0.5*(1-t4)*(p4^2 + p9^2)
  class  = sum t4 * sum_c (p_c-t_c)^2, c=10..29
  loss = (coord+conf+noobj+class)/N
All masked squares use (mask*e)^2 = mask*e^2 (masks binary); weights fold
into the ACT Square scale (sqrt(5), sqrt(0.5)). Every reduction rides the
fused accum_out, so each core emits a [128, NSLOT] partial tile that the
host sums.

Layout per core: 512 samples -> 2 passes x (128 partitions x 2 groups).
Channel c of a pass is the [128, 2, 196] slice at free offset c*196.
"""

import math

import numpy as np

import concourse.mybir as mybir
from concourse import bacc
from concourse.bass_utils import run_bass_kernel_spmd
from concourse.tile import TileContext

F32 = mybir.dt.float32
OP = mybir.AluOpType
AF = mybir.ActivationFunctionType

N, D, S = 4096, 30, 14
SS = S * S          # 196
NCORE = 8
NPC = N // NCORE    # 512 samples per core
P = 128
NPASS = 2
GRP = NPC // (NPASS * P)     # 2 groups per pass
CLS_CHUNK = 4                # class channels per chunk
NCHUNK = 20 // CLS_CHUNK     # 5 chunks
SLOTS_PER_PASS = 2 + 2 + 2 + NCHUNK   # coord + conf + noobj + class
NSLOT = SLOTS_PER_PASS * NPASS        # 34

_CACHE = {}


def _build():
    nc = bacc.Bacc("TRN2", target_bir_lowering=False, debug=False)
    pred = nc.dram_tensor("pred", [NPC, D * SS], F32, kind="ExternalInput")
    tgt = nc.dram_tensor("target", [NPC, D * SS], F32, kind="ExternalInput")
    out = nc.dram_tensor("out", [P, NSLOT], F32, kind="ExternalOutput")

    # [NPC, D*SS] -> [pass, partition, group, D*SS]
    pred_r = pred[:, :].rearrange("(q g p) d -> q p g d", q=NPASS, g=GRP, p=P)
    tgt_r = tgt[:, :].rearrange("(q g p) d -> q p g d", q=NPASS, g=GRP, p=P)

    sq5 = math.sqrt(5.0)
    sqh = math.sqrt(0.5)

    with TileContext(nc) as tc:
        with (
            tc.tile_pool(name="big", bufs=2) as big,      # box-channel data
            tc.tile_pool(name="cls", bufs=2) as clsp,     # class chunks
            tc.tile_pool(name="tmp", bufs=1) as tmp,      # short-lived temps
            tc.tile_pool(name="keep", bufs=2) as keep,    # pass-lived temps
            tc.tile_pool(name="accp", bufs=1) as accp,
        ):
            acc = accp.tile([P, NSLOT], F32)

            for q in range(NPASS):
                base = q * SLOTS_PER_PASS

                def slot(i):
                    return acc[:, base + i : base + i + 1]

                pb = big.tile([P, GRP, 10 * SS], F32, tag="pb", name="pb")
                tb = big.tile([P, GRP, 9 * SS], F32, tag="tb", name="tb")
                nc.sync.dma_start(out=pb, in_=pred_r[q, :, :, 0 : 10 * SS])
                nc.sync.dma_start(out=tb, in_=tgt_r[q, :, :, 0 : 9 * SS])

                def pch(c):
                    return pb[:, :, c * SS : (c + 1) * SS]

                def tch(c):
                    return tb[:, :, c * SS : (c + 1) * SS]

                t4 = tch(4)

                def T(tag, bufs=None):
                    return tmp.tile([P, GRP, SS], F32, tag=tag, name=tag,
                                    bufs=bufs)

                # ---- target box corners (shared by both boxes) ----
                thwx = T("thwx")
                thwy = T("thwy")
                nc.scalar.mul(thwx, tch(2), 0.5)
                nc.scalar.mul(thwy, tch(3), 0.5)
                tltx = keep.tile([P, GRP, SS], F32, tag="tltx", name="tltx")
                trbx = keep.tile([P, GRP, SS], F32, tag="trbx", name="trbx")
                tlty = keep.tile([P, GRP, SS], F32, tag="tlty", name="tlty")
                trby = keep.tile([P, GRP, SS], F32, tag="trby", name="trby")
                nc.vector.scalar_tensor_tensor(
                    tltx, tch(0), 1.0 / S, thwx, OP.mult, OP.subtract)
                nc.vector.scalar_tensor_tensor(
                    trbx, tch(0), 1.0 / S, thwx, OP.mult, OP.add)
                nc.vector.scalar_tensor_tensor(
                    tlty, tch(1), 1.0 / S, thwy, OP.mult, OP.subtract)
                nc.vector.scalar_tensor_tensor(
                    trby, tch(1), 1.0 / S, thwy, OP.mult, OP.add)
                tarea = keep.tile([P, GRP, SS], F32, tag="tarea", name="tarea")
                nc.vector.tensor_mul(tarea, tch(2), tch(3))

                # ---- per-box IoU ----
                ious = []
                for b in range(2):
                    o = 5 * b
                    phwx = T("phwx")
                    phwy = T("phwy")
                    nc.scalar.mul(phwx, pch(o + 2), 0.5)
                    nc.scalar.mul(phwy, pch(o + 3), 0.5)
                    pltx = T("pltx")
                    prbx = T("prbx")
                    plty = T("plty")
                    prby = T("prby")
                    nc.vector.scalar_tensor_tensor(
                        pltx, pch(o), 1.0 / S, phwx, OP.mult, OP.subtract)
                    nc.vector.scalar_tensor_tensor(
                        prbx, pch(o), 1.0 / S, phwx, OP.mult, OP.add)
                    nc.vector.scalar_tensor_tensor(
                        plty, pch(o + 1), 1.0 / S, phwy, OP.mult, OP.subtract)
                    nc.vector.scalar_tensor_tensor(
                        prby, pch(o + 1), 1.0 / S, phwy, OP.mult, OP.add)
                    ltx = T("ltx")
                    rbx = T("rbx")
                    lty = T("lty")
                    rby = T("rby")
                    nc.vector.tensor_max(ltx, tltx, pltx)
                    nc.vector.tensor_tensor(rbx, trbx, prbx, OP.min)
                    nc.vector.tensor_max(lty, tlty, plty)
                    nc.vector.tensor_tensor(rby, trby, prby, OP.min)
                    ox = T("ox")
                    oy = T("oy")
                    nc.vector.tensor_sub(ox, rbx, ltx)
                    nc.vector.tensor_sub(oy, rby, lty)
                    oyr = T("oyr")
                    nc.scalar.activation(oyr, oy, AF.Relu)
                    inter = T(f"inter{b}")
                    nc.vector.scalar_tensor_tensor(
                        inter, ox, 0.0, oyr, OP.max, OP.mult)
                    parea = T("parea")
                    nc.vector.tensor_mul(parea, pch(o + 2), pch(o + 3))
                    s1 = T("s1")
                    nc.vector.tensor_add(s1, parea, tarea)
                    union = T("union")
                    nc.vector.scalar_tensor_tensor(
                        union, inter, -1.0, s1, OP.mult, OP.add)
                    usafe = T("usafe")
                    nc.vector.scalar_tensor_tensor(
                        usafe, union, 0.0, union, OP.is_equal, OP.add)
                    r = T("recip")
                    nc.vector.reciprocal_approx_fast(out=r, in_=usafe)
                    iou = keep.tile([P, GRP, SS], F32, tag=f"iou{b}", name=f"iou{b}")
                    nc.vector.tensor_mul(iou, inter, r)
                    ious.append(iou)

                # ---- responsible-box masks ----
                sel = T("sel")
                nc.vector.tensor_tensor(sel, ious[1], ious[0], OP.is_gt)
                selm = keep.tile([P, GRP, SS], F32, tag="selm", name="selm")
                s0m = keep.tile([P, GRP, SS], F32, tag="s0m", name="s0m")
                nc.vector.tensor_mul(selm, sel, t4)
                nc.vector.tensor_sub(s0m, t4, selm)
                t4rep = keep.tile([P, GRP, CLS_CHUNK * SS], F32, tag="t4rep",
                                  name="t4rep", bufs=1)
                for c in range(CLS_CHUNK):
                    nc.scalar.copy(t4rep[:, :, c * SS:(c + 1) * SS], t4)

                # ---- coord terms: 4 contiguous channels fused,
                # masks replicated across the 4 channels ----
                selm_rep = keep.tile([P, GRP, 4 * SS], F32, tag="selm_rep",
                                     name="selm_rep", bufs=1)
                for c in range(4):
                    nc.scalar.copy(selm_rep[:, :, c * SS:(c + 1) * SS], selm)
                s0m_rep = keep.tile([P, GRP, 4 * SS], F32, tag="s0m_rep",
                                    name="s0m_rep", bufs=1)
                nc.vector.tensor_sub(s0m_rep, t4rep[:, :, 0:4 * SS], selm_rep)
                si = 0
                for h, mrep in ((0, s0m_rep), (1, selm_rep)):
                    lo = (5 * h) * SS
                    e4 = tmp.tile([P, GRP, 4 * SS], F32, tag="e4", name="e4",
                                  bufs=2)
                    me4 = tmp.tile([P, GRP, 4 * SS], F32, tag="me4", name="me4",
                                   bufs=2)
                    nc.gpsimd.tensor_sub(
                        e4, pb[:, :, lo:lo + 4 * SS], tb[:, :, lo:lo + 4 * SS])
                    nc.vector.tensor_mul(me4, e4, mrep)
                    nc.scalar.activation(
                        me4, me4, AF.Square, scale=sq5, accum_out=slot(si))
                    si += 1

                # ---- conf terms ----
                for h, mask in ((0, s0m), (1, selm)):
                    f = tmp.tile([P, GRP, SS], F32, tag="f", name="f", bufs=2)
                    mf = tmp.tile([P, GRP, SS], F32, tag="mf", name="mf", bufs=2)
                    nc.vector.tensor_sub(f, pch(4 + 5 * h), ious[h])
                    nc.gpsimd.tensor_mul(mf, f, mask)
                    nc.scalar.activation(
                        mf, mf, AF.Square, scale=1.0, accum_out=slot(si))
                    si += 1

                # ---- noobj terms ----
                w = keep.tile([P, GRP, SS], F32, tag="w", name="w")
                nc.scalar.activation(w, t4, AF.Copy, bias=1.0, scale=-1.0)
                for c in (4, 9):
                    m = tmp.tile([P, GRP, SS], F32, tag="m", name="m", bufs=2)
                    nc.vector.tensor_mul(m, pch(c), w)
                    nc.scalar.activation(
                        m, m, AF.Square, scale=sqh, accum_out=slot(si))
                    si += 1

                # ---- class terms ----
                for j in range(NCHUNK):
                    lo = (10 + CLS_CHUNK * j) * SS
                    hi = lo + CLS_CHUNK * SS
                    pc = clsp.tile([P, GRP, CLS_CHUNK * SS], F32, tag="pc", name="pc")
                    tcl = clsp.tile([P, GRP, CLS_CHUNK * SS], F32, tag="tc", name="tc")
                    nc.sync.dma_start(out=pc, in_=pred_r[q, :, :, lo:hi])
                    nc.sync.dma_start(out=tcl, in_=tgt_r[q, :, :, lo:hi])
                    nc.gpsimd.tensor_sub(pc, pc, tcl)
                    mul_eng = nc.gpsimd if j >= 3 else nc.vector
                    mul_eng.tensor_mul(pc, pc, t4rep)
                    nc.scalar.activation(
                        pc, pc, AF.Square, scale=1.0, accum_out=slot(si))
                    si += 1
                assert si == SLOTS_PER_PASS

            nc.sync.dma_start(out=out[:, :], in_=acc)
    nc.compile()
    return nc


def _get_nc():
    if "nc" not in _CACHE:
        _CACHE["nc"] = _build()
    return _CACHE["nc"]


def kernel(pred: np.ndarray, target: np.ndarray) -> np.ndarray:
    nc = _get_nc()
    in_maps = []
    for k in range(NCORE):
        sl = slice(k * NPC, (k + 1) * NPC)
        in_maps.append({
            "pred": np.ascontiguousarray(pred[sl]).reshape(NPC, D * SS),
            "target": np.ascontiguousarray(target[sl]).reshape(NPC, D * SS),
        })
    res = run_bass_kernel_spmd(nc, in_maps, core_ids=list(range(NCORE)))
    total = sum(float(r["out"].astype(np.float64).sum()) for r in res.results)
    return np.float32(total / N)



# revision 5
# speedup vs baseline: 1.3594x; 1.3594x over previous
"""YOLO loss kernel for Trainium2 (Bass/Tile), data-parallel over 8 NeuronCores.

Math (per sample n, cell s; S=14, SS=196, B=2, C=20, D=30):
  t4 = target conf channel (binary 0/1, channels 4 and 9 identical)
  All box coords scaled by S (iou is invariant): corner = x +- (S/2)w,
  areas = S^2 * w * h.
  For the box PAIR (channels {o..o+3} for o in {0,5}, strided APs):
    prb = x + (S/2)w ; nlt = (S/2)w - x   (= -lt)
    ox = min(tprb, prb) + min(tnlt, nlt) ; oy likewise
    inter = relu(ox)*max(oy,0) ; union = S^2*pw*ph + S^2*tw*th - inter
    iou = inter * recip(union)
  sel = iou1 > iou0 ; selm = sel*t4 ; s0m = t4 - selm  (mask pair msk=[s0m,selm])
  conf  = sum msk_h*(p_{4+5h} - iou_h)^2          (pair op, channels {4,9})
  noobj = sum 0.5*(1-t4)*(p4^2 + p9^2)            (pair op)
  coord = 5 * sum_k msk_h*(p_{5h+k}-t_{5h+k})^2   (8-channel pair op, bcast msk)
  class = sum t4 * (p_c-t_c)^2, c=10..29          (4-channel chunks, bcast t4)
Masked squares use (mask*e)^2 = mask*e^2 (masks binary); weights fold into the
ACT Square scale. Every reduction is an ACT accum_out into a [128, NSLOT]
partial tile; host sums across slots/partitions/cores and divides by N.

Layout per core: 512 samples = 4 blocks x 128 partitions, processed in passes
of GRPS=[1,2,1] blocks (small first pass -> compute starts early; small last
pass + split last class chunk -> short drain tail). Engine split per the
TimelineSim cost model: DVE all stt/min/cmp + chain ops, Pool (gpsimd) the
independent big sub/mults, ACT all square-accumulate reductions.
"""

import numpy as np

import concourse.mybir as mybir
from concourse import bacc
from concourse.bass_utils import run_bass_kernel_spmd
from concourse.tile import TileContext

F32 = mybir.dt.float32
OP = mybir.AluOpType
AF = mybir.ActivationFunctionType

N, D, S = 4096, 30, 14
SS = S * S          # 196
NCORE = 8
NPC = N // NCORE    # 512 samples per core
P = 128
NBLK = NPC // P     # 4 blocks of 128 samples
GRPS = [1, 2, 1]    # blocks per pass
HALF = S / 2.0      # corner scale
AREA = float(S * S)
SQ5 = 5.0 ** 0.5
SQH = 0.5 ** 0.5

# class chunks (channel counts); last pass splits the tail chunk
CLS_CHUNKS = [[4, 4, 4, 4, 4], [4, 4, 4, 4, 4], [4, 4, 4, 4, 2, 2]]
SLOTS_PER_PASS = [3 + len(c) for c in CLS_CHUNKS]   # noobj, conf, coord + class
NSLOT = sum(SLOTS_PER_PASS)                          # 25

_CACHE = {}


def _build():
    nc = bacc.Bacc("TRN2", target_bir_lowering=False, debug=False)
    pred = nc.dram_tensor("pred", [NPC, D * SS], F32, kind="ExternalInput")
    tgt = nc.dram_tensor("target", [NPC, D * SS], F32, kind="ExternalInput")
    out = nc.dram_tensor("out", [P, NSLOT], F32, kind="ExternalOutput")

    # [NPC, D*SS] -> [P, block, D*SS]; sample = block*128 + p
    pred_r = pred[:, :].rearrange("(a p) d -> p a d", a=NBLK)
    tgt_r = tgt[:, :].rearrange("(a p) d -> p a d", a=NBLK)

    npass = len(GRPS)
    offs = [sum(GRPS[:q]) for q in range(npass)]

    with TileContext(nc) as tc:
        with (
            tc.tile_pool(name="box", bufs=1) as boxp,     # pb/tb per pass
            tc.tile_pool(name="cls", bufs=3) as clsp,     # class chunk streams
            tc.tile_pool(name="tmp", bufs=1) as tmp,      # per-pass temps
            tc.tile_pool(name="accp", bufs=1) as accp,
        ):
            acc = accp.tile([P, NSLOT], F32)

            pb, tb = [], []
            for q, g in enumerate(GRPS):
                a0 = offs[q]
                pbq = boxp.tile([P, g, 10, SS], F32, tag=f"pb{q}", name=f"pb{q}")
                tbq = boxp.tile([P, g, 10, SS], F32, tag=f"tb{q}", name=f"tb{q}")
                # tb padded to 10-channel stride so {0..3,5..8} is a regular AP
                nc.sync.dma_start(
                    out=tbq[:, :, 0:9, :], in_=tgt_r[:, a0:a0 + g, 0:9 * SS])
                nc.sync.dma_start(
                    out=pbq, in_=pred_r[:, a0:a0 + g, 0:10 * SS])
                pb.append(pbq)
                tb.append(tbq)

            # class chunk DMAs (SP queue, streamed after box data)
            pc, tcl = [], []
            for q, g in enumerate(GRPS):
                a0 = offs[q]
                pcq, tcq = [], []
                lo = 10
                for j, w in enumerate(CLS_CHUNKS[q]):
                    pj = clsp.tile([P, g, w, SS], F32, tag=f"pc{g}_{w}",
                                   name=f"pc{q}_{j}")
                    tj = clsp.tile([P, g, w, SS], F32, tag=f"tc{g}_{w}",
                                   name=f"tc{q}_{j}")
                    nc.sync.dma_start(
                        out=pj, in_=pred_r[:, a0:a0 + g, lo * SS:(lo + w) * SS])
                    nc.sync.dma_start(
                        out=tj, in_=tgt_r[:, a0:a0 + g, lo * SS:(lo + w) * SS])
                    pcq.append(pj)
                    tcq.append(tj)
                    lo += w
                pc.append(pcq)
                tcl.append(tcq)

            # ---- per-pass state ----
            st = [dict() for _ in range(npass)]

            def slot(q, i):
                base = sum(SLOTS_PER_PASS[:q])
                return acc[:, base + i:base + i + 1]

            def T(q, name, shape):
                t = tmp.tile(shape, F32, tag=f"{name}g{GRPS[q]}",
                             name=f"{name}{q}")
                st[q][name] = t
                return t

            def box_phase(nc, q):
                g = GRPS[q]
                s = st[q]
                pbv = pb[q][:, :, :, :]
                tbv = tb[q][:, :, :, :]
                t4 = tbv[:, :, 4, :]

                # target prep (DVE stt): scaled corners + area
                tpr = T(q, "tpr", [P, g, 2, SS])    # [x-axis, y-axis]
                tnl = T(q, "tnl", [P, g, 2, SS])
                nc.vector.scalar_tensor_tensor(
                    tpr, tbv[:, :, 2:4, :], HALF, tbv[:, :, 0:2, :],
                    OP.mult, OP.add)
                nc.vector.scalar_tensor_tensor(
                    tnl, tbv[:, :, 2:4, :], HALF, tbv[:, :, 0:2, :],
                    OP.mult, OP.subtract)
                tarea = T(q, "tarea", [P, g, SS])
                nc.vector.scalar_tensor_tensor(
                    tarea, tbv[:, :, 2, :], AREA, tbv[:, :, 3, :],
                    OP.mult, OP.mult)

                # w = 1 - t4 (ACT); noobj pair masked by w
                w = T(q, "w", [P, g, SS])
                nc.scalar.activation(w, t4, AF.Copy, bias=1.0, scale=-1.0)
                conf_p = pbv.rearrange("p g (h c) s -> p g h c s", h=2)[:, :, :, 4, :]
                nm = T(q, "ce", [P, g, 2, SS])
                wb2 = w[:, :, :].unsqueeze(2).broadcast_to([P, g, 2, SS])
                nc.gpsimd.tensor_tensor(nm, conf_p, wb2, OP.mult)
                nc.scalar.activation(nm, nm, AF.Square, scale=SQH,
                                     accum_out=slot(q, 0))

                # pred corners, both boxes at once: channel pairs {c, c+5}
                pv = pbv.rearrange("p g (h c) s -> p g h c s", h=2)
                pw_x = pv[:, :, :, 2, :]   # [P, g, 2(box), SS]
                pw_y = pv[:, :, :, 3, :]
                px = pv[:, :, :, 0, :]
                py = pv[:, :, :, 1, :]
                prx = T(q, "prx", [P, g, 2, SS])
                nlx = T(q, "nlx", [P, g, 2, SS])
                pry = T(q, "pry", [P, g, 2, SS])
                nly = T(q, "nly", [P, g, 2, SS])
                nc.vector.scalar_tensor_tensor(prx, pw_x, HALF, px, OP.mult, OP.add)
                nc.vector.scalar_tensor_tensor(nlx, pw_x, HALF, px, OP.mult, OP.subtract)
                nc.vector.scalar_tensor_tensor(pry, pw_y, HALF, py, OP.mult, OP.add)
                nc.vector.scalar_tensor_tensor(nly, pw_y, HALF, py, OP.mult, OP.subtract)
                # pq = pred area (Pool): (S^2 folded in s1 stt via AREA scale)
                pq = T(q, "pq", [P, g, 2, SS])
                nc.gpsimd.tensor_tensor(pq, pw_x, pw_y, OP.mult)

                # intersect: mins (target side broadcast over box pair)
                tprxb = tpr[:, :, 0, :].unsqueeze(2).broadcast_to([P, g, 2, SS])
                tpryb = tpr[:, :, 1, :].unsqueeze(2).broadcast_to([P, g, 2, SS])
                tnlxb = tnl[:, :, 0, :].unsqueeze(2).broadcast_to([P, g, 2, SS])
                tnlyb = tnl[:, :, 1, :].unsqueeze(2).broadcast_to([P, g, 2, SS])
                nc.vector.tensor_tensor(prx, tprxb, prx, OP.min)
                nc.vector.tensor_tensor(nlx, tnlxb, nlx, OP.min)
                nc.vector.tensor_tensor(pry, tpryb, pry, OP.min)
                nc.vector.tensor_tensor(nly, tnlyb, nly, OP.min)
                nc.vector.tensor_add(prx, prx, nlx)          # ox
                nc.vector.tensor_add(pry, pry, nly)          # oy
                nc.scalar.activation(nlx, prx, AF.Relu)      # relu(ox)
                nc.vector.scalar_tensor_tensor(
                    pry, pry, 0.0, nlx, OP.max, OP.mult)     # inter
                # union = (S^2*pq - inter) + tarea
                tareab = tarea[:, :, :].unsqueeze(2).broadcast_to([P, g, 2, SS])
                nc.vector.scalar_tensor_tensor(
                    nly, pq, AREA, pry, OP.mult, OP.subtract)
                nc.vector.tensor_tensor(nly, nly, tareab, OP.add)    # union
                nc.vector.reciprocal_approx_fast(
                    out=prx[:, :, :, :].rearrange("p g h s -> p (g h s)"),
                    in_=nly[:, :, :, :].rearrange("p g h s -> p (g h s)"))
                iou = pq
                st[q]["iou"] = iou
                nc.vector.tensor_mul(iou, pry, prx)

                # masks
                sel = tarea
                nc.vector.tensor_tensor(
                    sel, iou[:, :, 1, :], iou[:, :, 0, :], OP.is_gt)
                msk = T(q, "msk", [P, g, 2, SS])
                nc.vector.tensor_mul(msk[:, :, 1, :], sel, t4)       # selm
                nc.vector.tensor_sub(msk[:, :, 0, :], t4, msk[:, :, 1, :])  # s0m

                # conf pair: (p_conf - iou) * msk
                ce = s["ce"]
                nc.vector.tensor_sub(ce, conf_p, iou)
                nc.vector.tensor_mul(ce, ce, msk)
                nc.scalar.activation(ce, ce, AF.Square, scale=1.0,
                                     accum_out=slot(q, 1))

            def coord_phase(nc, q, sub_eng, mul_eng):
                g = GRPS[q]
                s = st[q]
                pv8 = pb[q][:, :, :, :].rearrange(
                    "p g (h c) s -> p g h c s", h=2)[:, :, :, 0:4, :]
                tv8 = tb[q][:, :, :, :].rearrange(
                    "p g (h c) s -> p g h c s", h=2)[:, :, :, 0:4, :]
                e8 = T(q, "e8", [P, g, 2, 4, SS])
                sub_eng.tensor_tensor(e8, pv8, tv8, OP.subtract)
                mskb = s["msk"][:, :, :, :].unsqueeze(3).broadcast_to(
                    [P, g, 2, 4, SS])
                mul_eng.tensor_tensor(e8, e8, mskb, OP.mult)
                nc.scalar.activation(e8, e8, AF.Square, scale=SQ5,
                                     accum_out=slot(q, 2))

            def class_chunk(nc, q, j, sub_eng, mul_eng):
                g = GRPS[q]
                w = CLS_CHUNKS[q][j]
                t4 = tb[q][:, :, :, :][:, :, 4, :]
                e = pc[q][j]
                sub_eng.tensor_tensor(e, e, tcl[q][j], OP.subtract)
                t4b = t4.unsqueeze(2).broadcast_to([P, g, w, SS])
                mul_eng.tensor_tensor(e, e, t4b, OP.mult)
                nc.scalar.activation(e, e, AF.Square, scale=1.0,
                                     accum_out=slot(q, 3 + j))

            V, G = nc.vector, nc.gpsimd

            # ---- emission order (per-engine queues are in-order) ----
            box_phase(nc, 0)
            box_phase(nc, 1)
            coord_phase(nc, 0, G, V)
            box_phase(nc, 2)
            coord_phase(nc, 1, G, V)
            coord_phase(nc, 2, G, V)
            for j in range(len(CLS_CHUNKS[0])):
                class_chunk(nc, 0, j, G, V)
            for j in range(len(CLS_CHUNKS[1])):
                class_chunk(nc, 1, j, G, V)
            for j in range(len(CLS_CHUNKS[2])):
                # short tail: last two (2-channel) chunks all-DVE
                eng = (G, V) if j < 4 else (V, V)
                class_chunk(nc, 2, j, *eng)

            nc.sync.dma_start(out=out[:, :], in_=acc)
    nc.compile()
    return nc


def _get_nc():
    if "nc" not in _CACHE:
        _CACHE["nc"] = _build()
    return _CACHE["nc"]


def kernel(pred: np.ndarray, target: np.ndarray) -> np.ndarray:
    nc = _get_nc()
    in_maps = []
    for k in range(NCORE):
        sl = slice(k * NPC, (k + 1) * NPC)
        in_maps.append({
            "pred": np.ascontiguousarray(pred[sl]).reshape(NPC, D * SS),
            "target": np.ascontiguousarray(target[sl]).reshape(NPC, D * SS),
        })
    res = run_bass_kernel_spmd(nc, in_maps, core_ids=list(range(NCORE)))
    total = sum(float(r["out"].astype(np.float64).sum()) for r in res.results)
    return np.float32(total / N)


# revision 6
# speedup vs baseline: 1.3619x; 1.0018x over previous
"""YOLO loss kernel for Trainium2 (Bass/Tile), data-parallel over 8 NeuronCores.

Math (per sample n, cell s; S=14, SS=196, B=2, C=20, D=30):
  t4 = target conf channel (binary 0/1, channels 4 and 9 identical)
  All box coords scaled by S (iou is invariant): corner = x +- (S/2)w,
  areas = S^2 * w * h.
  For the box PAIR (channels {o..o+3} for o in {0,5}, strided APs):
    prb = x + (S/2)w ; nlt = (S/2)w - x   (= -lt)
    ox = min(tprb, prb) + min(tnlt, nlt) ; oy likewise
    inter = relu(ox)*max(oy,0) ; union = S^2*pw*ph + S^2*tw*th - inter
    iou = inter * recip(union)
  sel = iou1 > iou0 ; selm = sel*t4 ; s0m = t4 - selm  (mask pair msk=[s0m,selm])
  conf  = sum msk_h*(p_{4+5h} - iou_h)^2          (pair op, channels {4,9})
  noobj = sum 0.5*(1-t4)*(p4^2 + p9^2)            (pair op)
  coord = 5 * sum_k msk_h*(p_{5h+k}-t_{5h+k})^2   (8-channel pair op, bcast msk)
  class = sum t4 * (p_c-t_c)^2, c=10..29          (4-channel chunks, bcast t4)
Masked squares use (mask*e)^2 = mask*e^2 (masks binary); weights fold into the
ACT Square scale. Every reduction is an ACT accum_out into a [128, NSLOT]
partial tile; host sums across slots/partitions/cores and divides by N.

Layout per core: 512 samples = 4 blocks x 128 partitions, processed in passes
of GRPS=[1,2,1] blocks (small first pass -> compute starts early; small last
pass + split last class chunk -> short drain tail). Engine split per the
TimelineSim cost model: DVE all stt/min/cmp + chain ops, Pool (gpsimd) the
independent big sub/mults, ACT all square-accumulate reductions.
"""

import numpy as np

import concourse.mybir as mybir
from concourse import bacc
from concourse.bass_utils import run_bass_kernel_spmd
from concourse.tile import TileContext

F32 = mybir.dt.float32
OP = mybir.AluOpType
AF = mybir.ActivationFunctionType

N, D, S = 4096, 30, 14
SS = S * S          # 196
NCORE = 8
NPC = N // NCORE    # 512 samples per core
P = 128
NBLK = NPC // P     # 4 blocks of 128 samples
GRPS = [1, 2, 1]    # blocks per pass
HALF = S / 2.0      # corner scale
AREA = float(S * S)
SQ5 = 5.0 ** 0.5
SQH = 0.5 ** 0.5

# class chunks (channel counts); last pass splits the tail chunk
CLS_CHUNKS = [[4, 4, 4, 4, 4], [4, 4, 4, 4, 4], [4, 4, 4, 4, 2, 1, 1]]
SLOTS_PER_PASS = [3 + len(c) for c in CLS_CHUNKS]   # noobj, conf, coord + class
NSLOT = sum(SLOTS_PER_PASS)                          # 25

_CACHE = {}


def _build():
    nc = bacc.Bacc("TRN2", target_bir_lowering=False, debug=False)
    pred = nc.dram_tensor("pred", [NPC, D * SS], F32, kind="ExternalInput")
    tgt = nc.dram_tensor("target", [NPC, D * SS], F32, kind="ExternalInput")
    out = nc.dram_tensor("out", [P, NSLOT], F32, kind="ExternalOutput")

    # [NPC, D*SS] -> [P, block, D*SS]; sample = block*128 + p
    pred_r = pred[:, :].rearrange("(a p) d -> p a d", a=NBLK)
    tgt_r = tgt[:, :].rearrange("(a p) d -> p a d", a=NBLK)

    npass = len(GRPS)
    offs = [sum(GRPS[:q]) for q in range(npass)]

    with TileContext(nc) as tc:
        with (
            tc.tile_pool(name="box", bufs=1) as boxp,     # pb/tb per pass
            tc.tile_pool(name="cls", bufs=3) as clsp,     # class chunk streams
            tc.tile_pool(name="tmp", bufs=1) as tmp,      # per-pass temps
            tc.tile_pool(name="accp", bufs=1) as accp,
        ):
            acc = accp.tile([P, NSLOT], F32)

            pb, tb = [], []
            for q, g in enumerate(GRPS):
                a0 = offs[q]
                pbq = boxp.tile([P, g, 10, SS], F32, tag=f"pb{q}", name=f"pb{q}")
                tbq = boxp.tile([P, g, 10, SS], F32, tag=f"tb{q}", name=f"tb{q}")
                # tb padded to 10-channel stride so {0..3,5..8} is a regular AP
                nc.sync.dma_start(
                    out=tbq[:, :, 0:9, :], in_=tgt_r[:, a0:a0 + g, 0:9 * SS])
                nc.sync.dma_start(
                    out=pbq, in_=pred_r[:, a0:a0 + g, 0:10 * SS])
                pb.append(pbq)
                tb.append(tbq)

            # class chunk DMAs (SP queue, streamed after box data)
            pc, tcl = [], []
            for q, g in enumerate(GRPS):
                a0 = offs[q]
                pcq, tcq = [], []
                lo = 10
                for j, w in enumerate(CLS_CHUNKS[q]):
                    pj = clsp.tile([P, g, w, SS], F32, tag=f"pc{g}_{w}",
                                   name=f"pc{q}_{j}", bufs=3 if w == 4 else 2)
                    tj = clsp.tile([P, g, w, SS], F32, tag=f"tc{g}_{w}",
                                   name=f"tc{q}_{j}", bufs=4 if w == 4 else 2)
                    nc.sync.dma_start(
                        out=pj, in_=pred_r[:, a0:a0 + g, lo * SS:(lo + w) * SS])
                    nc.sync.dma_start(
                        out=tj, in_=tgt_r[:, a0:a0 + g, lo * SS:(lo + w) * SS])
                    pcq.append(pj)
                    tcq.append(tj)
                    lo += w
                pc.append(pcq)
                tcl.append(tcq)

            # ---- per-pass state ----
            st = [dict() for _ in range(npass)]

            def slot(q, i):
                base = sum(SLOTS_PER_PASS[:q])
                return acc[:, base + i:base + i + 1]

            def T(q, name, shape):
                t = tmp.tile(shape, F32, tag=f"{name}g{GRPS[q]}",
                             name=f"{name}{q}")
                st[q][name] = t
                return t

            def box_phase(nc, q):
                g = GRPS[q]
                s = st[q]
                pbv = pb[q][:, :, :, :]
                tbv = tb[q][:, :, :, :]
                t4 = tbv[:, :, 4, :]

                # target prep (DVE stt): scaled corners + area
                tpr = T(q, "tpr", [P, g, 2, SS])    # [x-axis, y-axis]
                tnl = T(q, "tnl", [P, g, 2, SS])
                nc.vector.scalar_tensor_tensor(
                    tpr, tbv[:, :, 2:4, :], HALF, tbv[:, :, 0:2, :],
                    OP.mult, OP.add)
                nc.vector.scalar_tensor_tensor(
                    tnl, tbv[:, :, 2:4, :], HALF, tbv[:, :, 0:2, :],
                    OP.mult, OP.subtract)
                tarea = T(q, "tarea", [P, g, SS])
                nc.vector.scalar_tensor_tensor(
                    tarea, tbv[:, :, 2, :], AREA, tbv[:, :, 3, :],
                    OP.mult, OP.mult)

                # w = 1 - t4 (ACT); noobj pair masked by w
                w = T(q, "w", [P, g, SS])
                nc.scalar.activation(w, t4, AF.Copy, bias=1.0, scale=-1.0)
                conf_p = pbv.rearrange("p g (h c) s -> p g h c s", h=2)[:, :, :, 4, :]
                nm = T(q, "ce", [P, g, 2, SS])
                wb2 = w[:, :, :].unsqueeze(2).broadcast_to([P, g, 2, SS])
                nc.gpsimd.tensor_tensor(nm, conf_p, wb2, OP.mult)
                nc.scalar.activation(nm, nm, AF.Square, scale=SQH,
                                     accum_out=slot(q, 0))

                # pred corners, both boxes at once: channel pairs {c, c+5}
                pv = pbv.rearrange("p g (h c) s -> p g h c s", h=2)
                pw_x = pv[:, :, :, 2, :]   # [P, g, 2(box), SS]
                pw_y = pv[:, :, :, 3, :]
                px = pv[:, :, :, 0, :]
                py = pv[:, :, :, 1, :]
                prx = T(q, "prx", [P, g, 2, SS])
                nlx = T(q, "nlx", [P, g, 2, SS])
                pry = T(q, "pry", [P, g, 2, SS])
                nly = T(q, "nly", [P, g, 2, SS])
                nc.vector.scalar_tensor_tensor(prx, pw_x, HALF, px, OP.mult, OP.add)
                nc.vector.scalar_tensor_tensor(nlx, pw_x, HALF, px, OP.mult, OP.subtract)
                nc.vector.scalar_tensor_tensor(pry, pw_y, HALF, py, OP.mult, OP.add)
                nc.vector.scalar_tensor_tensor(nly, pw_y, HALF, py, OP.mult, OP.subtract)
                # pq = pred area (Pool): (S^2 folded in s1 stt via AREA scale)
                pq = T(q, "pq", [P, g, 2, SS])
                nc.gpsimd.tensor_tensor(pq, pw_x, pw_y, OP.mult)

                # intersect: mins (target side broadcast over box pair)
                tprxb = tpr[:, :, 0, :].unsqueeze(2).broadcast_to([P, g, 2, SS])
                tpryb = tpr[:, :, 1, :].unsqueeze(2).broadcast_to([P, g, 2, SS])
                tnlxb = tnl[:, :, 0, :].unsqueeze(2).broadcast_to([P, g, 2, SS])
                tnlyb = tnl[:, :, 1, :].unsqueeze(2).broadcast_to([P, g, 2, SS])
                nc.vector.tensor_tensor(prx, tprxb, prx, OP.min)
                nc.vector.tensor_tensor(nlx, tnlxb, nlx, OP.min)
                nc.vector.tensor_tensor(pry, tpryb, pry, OP.min)
                nc.vector.tensor_tensor(nly, tnlyb, nly, OP.min)
                nc.vector.tensor_add(prx, prx, nlx)          # ox
                nc.vector.tensor_add(pry, pry, nly)          # oy
                nc.scalar.activation(nlx, prx, AF.Relu)      # relu(ox)
                nc.vector.scalar_tensor_tensor(
                    pry, pry, 0.0, nlx, OP.max, OP.mult)     # inter
                # union = (S^2*pq - inter) + tarea
                tareab = tarea[:, :, :].unsqueeze(2).broadcast_to([P, g, 2, SS])
                nc.vector.scalar_tensor_tensor(
                    nly, pq, AREA, pry, OP.mult, OP.subtract)
                nc.vector.tensor_tensor(nly, nly, tareab, OP.add)    # union
                nc.vector.reciprocal_approx_fast(
                    out=prx[:, :, :, :].rearrange("p g h s -> p (g h s)"),
                    in_=nly[:, :, :, :].rearrange("p g h s -> p (g h s)"))
                iou = pq
                st[q]["iou"] = iou
                nc.vector.tensor_mul(iou, pry, prx)

                # masks
                sel = tarea
                nc.vector.tensor_tensor(
                    sel, iou[:, :, 1, :], iou[:, :, 0, :], OP.is_gt)
                msk = T(q, "msk", [P, g, 2, SS])
                nc.vector.tensor_mul(msk[:, :, 1, :], sel, t4)       # selm
                nc.vector.tensor_sub(msk[:, :, 0, :], t4, msk[:, :, 1, :])  # s0m

                # conf pair: (p_conf - iou) * msk
                ce = s["ce"]
                nc.vector.tensor_sub(ce, conf_p, iou)
                nc.vector.tensor_mul(ce, ce, msk)
                nc.scalar.activation(ce, ce, AF.Square, scale=1.0,
                                     accum_out=slot(q, 1))

            def coord_phase(nc, q, sub_eng, mul_eng):
                g = GRPS[q]
                s = st[q]
                pv8 = pb[q][:, :, :, :].rearrange(
                    "p g (h c) s -> p g h c s", h=2)[:, :, :, 0:4, :]
                tv8 = tb[q][:, :, :, :].rearrange(
                    "p g (h c) s -> p g h c s", h=2)[:, :, :, 0:4, :]
                e8 = T(q, "e8", [P, g, 2, 4, SS])
                sub_eng.tensor_tensor(e8, pv8, tv8, OP.subtract)
                mskb = s["msk"][:, :, :, :].unsqueeze(3).broadcast_to(
                    [P, g, 2, 4, SS])
                mul_eng.tensor_tensor(e8, e8, mskb, OP.mult)
                nc.scalar.activation(e8, e8, AF.Square, scale=SQ5,
                                     accum_out=slot(q, 2))

            def class_chunk(nc, q, j, sub_eng, mul_eng):
                g = GRPS[q]
                w = CLS_CHUNKS[q][j]
                t4 = tb[q][:, :, :, :][:, :, 4, :]
                e = tcl[q][j]
                sub_eng.tensor_tensor(e, pc[q][j], e, OP.subtract)
                t4b = t4.unsqueeze(2).broadcast_to([P, g, w, SS])
                mul_eng.tensor_tensor(e, e, t4b, OP.mult)
                nc.scalar.activation(e, e, AF.Square, scale=1.0,
                                     accum_out=slot(q, 3 + j))

            V, G = nc.vector, nc.gpsimd

            # ---- emission order (per-engine queues are in-order),
            # ---- interleaved by expected readiness ----
            box_phase(nc, 0)
            box_phase(nc, 1)
            coord_phase(nc, 0, G, V)
            box_phase(nc, 2)
            class_chunk(nc, 0, 0, G, V)
            class_chunk(nc, 0, 1, G, V)
            coord_phase(nc, 1, G, V)
            class_chunk(nc, 0, 2, G, V)
            class_chunk(nc, 0, 3, G, V)
            coord_phase(nc, 2, G, V)
            class_chunk(nc, 0, 4, G, V)
            for j in range(len(CLS_CHUNKS[1])):
                sub_eng, mul_eng = (G, G) if j == 2 else (G, V)
                class_chunk(nc, 1, j, sub_eng, mul_eng)
            for j in range(len(CLS_CHUNKS[2])):
                # short tail: small trailing chunks all-DVE
                eng = (G, V) if j < 4 else (V, V)
                class_chunk(nc, 2, j, *eng)

            nc.sync.dma_start(out=out[:, :], in_=acc)
    nc.compile()
    return nc


def _get_nc():
    if "nc" not in _CACHE:
        _CACHE["nc"] = _build()
    return _CACHE["nc"]


def kernel(pred: np.ndarray, target: np.ndarray) -> np.ndarray:
    nc = _get_nc()
    in_maps = []
    for k in range(NCORE):
        sl = slice(k * NPC, (k + 1) * NPC)
        in_maps.append({
            "pred": np.ascontiguousarray(pred[sl]).reshape(NPC, D * SS),
            "target": np.ascontiguousarray(target[sl]).reshape(NPC, D * SS),
        })
    res = run_bass_kernel_spmd(nc, in_maps, core_ids=list(range(NCORE)))
    total = sum(float(r["out"].astype(np.float64).sum()) for r in res.results)
    return np.float32(total / N)


# revision 8
# speedup vs baseline: 1.4760x; 1.0838x over previous
"""YOLO loss kernel for Trainium2 (Bass/Tile), data-parallel over 8 NeuronCores.

Math (per sample n, cell s; S=14, SS=196, B=2, C=20, D=30):
  t4 = target conf channel (binary 0/1, channels 4 and 9 identical)
  All box coords scaled by S (iou is invariant): corner = x +- (S/2)w,
  areas = S^2 * w * h.
  For the box PAIR (channels {o..o+3} for o in {0,5}, strided APs):
    prb = x + (S/2)w ; nlt = (S/2)w - x   (= -lt)
    ox = min(tprb, prb) + min(tnlt, nlt) ; oy likewise
    inter = relu(ox)*max(oy,0) ; union = S^2*pw*ph + S^2*tw*th - inter
    iou = inter * recip(union)
  sel = iou1 > iou0 ; selm = sel*t4 ; s0m = t4 - selm  (mask pair msk=[s0m,selm])
  conf  = sum msk_h*(p_{4+5h} - iou_h)^2          (pair op, channels {4,9})
  noobj = sum 0.5*(1-t4)*(p4^2 + p9^2)            (pair op)
  coord = 5 * sum_k msk_h*(p_{5h+k}-t_{5h+k})^2   (8-channel pair op, bcast msk)
  class = sum t4 * (p_c-t_c)^2, c=10..29          (4-channel chunks, bcast t4)
Masked squares use (mask*e)^2 = mask*e^2 (masks binary); weights fold into the
ACT Square scale. Every reduction is an ACT accum_out into a [128, NSLOT]
partial tile; host sums across slots/partitions/cores and divides by N.

Layout per core: 512 samples = 4 blocks x 128 partitions, processed in passes
of GRPS=[1,2,1] blocks (small first pass -> compute starts early; small last
pass + split last class chunk -> short drain tail). Engine split per the
TimelineSim cost model: DVE all stt/min/cmp + chain ops, Pool (gpsimd) the
independent big sub/mults, ACT all square-accumulate reductions.
"""

import numpy as np

import concourse.mybir as mybir
from concourse import bacc
from concourse.bass_utils import run_bass_kernel_spmd
from concourse.tile import TileContext

F32 = mybir.dt.float32
OP = mybir.AluOpType
AF = mybir.ActivationFunctionType

N, D, S = 4096, 30, 14
SS = S * S          # 196
NCORE = 8
NPC = N // NCORE    # 512 samples per core
P = 128
NBLK = NPC // P     # 4 blocks of 128 samples
GRPS = [1, 2, 1]    # blocks per pass
HALF = S / 2.0      # corner scale
AREA = float(S * S)
SQ5 = 5.0 ** 0.5
SQH = 0.5 ** 0.5

# class chunks (channel counts); last pass splits the tail chunk
CLS_CHUNKS = [[4, 4, 4, 4, 4], [4, 4, 4, 4, 4], [4, 4, 4, 4, 2, 1, 1]]
SLOTS_PER_PASS = [3 + len(c) for c in CLS_CHUNKS]   # noobj, conf, coord + class
NSLOT = sum(SLOTS_PER_PASS)                          # 25

_CACHE = {}


def _build():
    nc = bacc.Bacc("TRN2", target_bir_lowering=False, debug=False)
    pred = nc.dram_tensor("pred", [NPC, D * SS], F32, kind="ExternalInput")
    tgt = nc.dram_tensor("target", [NPC, D * SS], F32, kind="ExternalInput")
    out = nc.dram_tensor("out", [P, NSLOT], F32, kind="ExternalOutput")

    # [NPC, D*SS] -> [P, block, D*SS]; sample = block*128 + p
    pred_r = pred[:, :].rearrange("(a p) d -> p a d", a=NBLK)
    tgt_r = tgt[:, :].rearrange("(a p) d -> p a d", a=NBLK)

    npass = len(GRPS)
    offs = [sum(GRPS[:q]) for q in range(npass)]

    with TileContext(nc) as tc:
        with (
            tc.tile_pool(name="box", bufs=1) as boxp,     # pb/tb per pass
            tc.tile_pool(name="cls", bufs=3) as clsp,     # class chunk streams
            tc.tile_pool(name="tmp", bufs=1) as tmp,      # per-pass temps
            tc.tile_pool(name="accp", bufs=1) as accp,
        ):
            acc = accp.tile([P, NSLOT], F32)

            pb, tb = [], []
            for q, g in enumerate(GRPS):
                a0 = offs[q]
                pbq = boxp.tile([P, g, 10, SS], F32, tag=f"pb{q}", name=f"pb{q}")
                tbq = boxp.tile([P, g, 10, SS], F32, tag=f"tb{q}", name=f"tb{q}")
                # tb padded to 10-channel stride so {0..3,5..8} is a regular AP
                nc.sync.dma_start(
                    out=tbq[:, :, 0:9, :], in_=tgt_r[:, a0:a0 + g, 0:9 * SS])
                nc.sync.dma_start(
                    out=pbq, in_=pred_r[:, a0:a0 + g, 0:10 * SS])
                pb.append(pbq)
                tb.append(tbq)

            # class chunk DMAs (SP queue, streamed after box data)
            pc, tcl = [], []
            for q, g in enumerate(GRPS):
                a0 = offs[q]
                pcq, tcq = [], []
                lo = 10
                for j, w in enumerate(CLS_CHUNKS[q]):
                    pbufs = (4 if g == 1 else 3) if w == 4 else 2
                    tbufs = 5 if w == 4 else 2
                    pj = clsp.tile([P, g, w, SS], F32, tag=f"pc{g}_{w}",
                                   name=f"pc{q}_{j}", bufs=pbufs)
                    tj = clsp.tile([P, g, w, SS], F32, tag=f"tc{g}_{w}",
                                   name=f"tc{q}_{j}", bufs=tbufs)
                    nc.sync.dma_start(
                        out=pj, in_=pred_r[:, a0:a0 + g, lo * SS:(lo + w) * SS])
                    nc.sync.dma_start(
                        out=tj, in_=tgt_r[:, a0:a0 + g, lo * SS:(lo + w) * SS])
                    pcq.append(pj)
                    tcq.append(tj)
                    lo += w
                pc.append(pcq)
                tcl.append(tcq)

            # ---- per-pass state ----
            st = [dict() for _ in range(npass)]

            def slot(q, i):
                base = sum(SLOTS_PER_PASS[:q])
                return acc[:, base + i:base + i + 1]

            def T(q, name, shape):
                t = tmp.tile(shape, F32, tag=f"{name}g{GRPS[q]}",
                             name=f"{name}{q}")
                st[q][name] = t
                return t

            def box_phase(nc, q):
                g = GRPS[q]
                s = st[q]
                pbv = pb[q][:, :, :, :]
                tbv = tb[q][:, :, :, :]
                t4 = tbv[:, :, 4, :]

                # target prep (DVE stt): scaled corners + area
                tpr = T(q, "tpr", [P, g, 2, SS])    # [x-axis, y-axis]
                tnl = T(q, "tnl", [P, g, 2, SS])
                nc.vector.scalar_tensor_tensor(
                    tpr, tbv[:, :, 2:4, :], HALF, tbv[:, :, 0:2, :],
                    OP.mult, OP.add)
                nc.vector.scalar_tensor_tensor(
                    tnl, tbv[:, :, 2:4, :], HALF, tbv[:, :, 0:2, :],
                    OP.mult, OP.subtract)
                tarea = T(q, "tarea", [P, g, SS])
                nc.vector.scalar_tensor_tensor(
                    tarea, tbv[:, :, 2, :], AREA, tbv[:, :, 3, :],
                    OP.mult, OP.mult)

                # w = 1 - t4 (ACT); noobj pair masked by w
                w = T(q, "w", [P, g, SS])
                nc.scalar.activation(w, t4, AF.Copy, bias=1.0, scale=-1.0)
                conf_p = pbv.rearrange("p g (h c) s -> p g h c s", h=2)[:, :, :, 4, :]
                nm = T(q, "ce", [P, g, 2, SS])
                wb2 = w[:, :, :].unsqueeze(2).broadcast_to([P, g, 2, SS])
                nc.gpsimd.tensor_tensor(nm, conf_p, wb2, OP.mult)
                nc.scalar.activation(nm, nm, AF.Square, scale=SQH,
                                     accum_out=slot(q, 0))

                # pred corners, both boxes at once: channel pairs {c, c+5}
                pv = pbv.rearrange("p g (h c) s -> p g h c s", h=2)
                pw_x = pv[:, :, :, 2, :]   # [P, g, 2(box), SS]
                pw_y = pv[:, :, :, 3, :]
                px = pv[:, :, :, 0, :]
                py = pv[:, :, :, 1, :]
                prx = T(q, "prx", [P, g, 2, SS])
                nlx = T(q, "nlx", [P, g, 2, SS])
                pry = T(q, "pry", [P, g, 2, SS])
                nly = T(q, "nly", [P, g, 2, SS])
                nc.vector.scalar_tensor_tensor(prx, pw_x, HALF, px, OP.mult, OP.add)
                nc.vector.scalar_tensor_tensor(nlx, pw_x, HALF, px, OP.mult, OP.subtract)
                nc.vector.scalar_tensor_tensor(pry, pw_y, HALF, py, OP.mult, OP.add)
                nc.vector.scalar_tensor_tensor(nly, pw_y, HALF, py, OP.mult, OP.subtract)
                # pq = pred area (Pool): (S^2 folded in s1 stt via AREA scale)
                pq = T(q, "pq", [P, g, 2, SS])
                nc.gpsimd.tensor_tensor(pq, pw_x, pw_y, OP.mult)

                # intersect: mins (target side broadcast over box pair)
                tprxb = tpr[:, :, 0, :].unsqueeze(2).broadcast_to([P, g, 2, SS])
                tpryb = tpr[:, :, 1, :].unsqueeze(2).broadcast_to([P, g, 2, SS])
                tnlxb = tnl[:, :, 0, :].unsqueeze(2).broadcast_to([P, g, 2, SS])
                tnlyb = tnl[:, :, 1, :].unsqueeze(2).broadcast_to([P, g, 2, SS])
                nc.vector.tensor_tensor(prx, tprxb, prx, OP.min)
                nc.vector.tensor_tensor(nlx, tnlxb, nlx, OP.min)
                nc.vector.tensor_tensor(pry, tpryb, pry, OP.min)
                nc.vector.tensor_tensor(nly, tnlyb, nly, OP.min)
                nc.vector.tensor_add(prx, prx, nlx)          # ox
                nc.vector.tensor_add(pry, pry, nly)          # oy
                nc.scalar.activation(nlx, prx, AF.Relu)      # relu(ox)
                nc.vector.scalar_tensor_tensor(
                    pry, pry, 0.0, nlx, OP.max, OP.mult)     # inter
                # union = (S^2*pq - inter) + tarea
                tareab = tarea[:, :, :].unsqueeze(2).broadcast_to([P, g, 2, SS])
                nc.vector.scalar_tensor_tensor(
                    nly, pq, AREA, pry, OP.mult, OP.subtract)
                nc.vector.tensor_tensor(nly, nly, tareab, OP.add)    # union
                nc.vector.reciprocal_approx_fast(
                    out=prx[:, :, :, :].rearrange("p g h s -> p (g h s)"),
                    in_=nly[:, :, :, :].rearrange("p g h s -> p (g h s)"))
                iou = pq
                st[q]["iou"] = iou
                nc.vector.tensor_mul(iou, pry, prx)

                # masks
                sel = tarea
                nc.vector.tensor_tensor(
                    sel, iou[:, :, 1, :], iou[:, :, 0, :], OP.is_gt)
                msk = s["tpr"]
                st[q]["msk"] = msk
                nc.vector.tensor_mul(msk[:, :, 1, :], sel, t4)       # selm
                nc.vector.tensor_sub(msk[:, :, 0, :], t4, msk[:, :, 1, :])  # s0m

                # conf pair: (p_conf - iou) * msk
                ce = s["ce"]
                nc.vector.tensor_sub(ce, conf_p, iou)
                nc.vector.tensor_mul(ce, ce, msk)
                nc.scalar.activation(ce, ce, AF.Square, scale=1.0,
                                     accum_out=slot(q, 1))

            def coord_phase(nc, q, sub_eng, mul_eng):
                g = GRPS[q]
                s = st[q]
                pv8 = pb[q][:, :, :, :].rearrange(
                    "p g (h c) s -> p g h c s", h=2)[:, :, :, 0:4, :]
                tv8 = tb[q][:, :, :, :].rearrange(
                    "p g (h c) s -> p g h c s", h=2)[:, :, :, 0:4, :]
                e8 = T(q, "e8", [P, g, 2, 4, SS])
                sub_eng.tensor_tensor(e8, pv8, tv8, OP.subtract)
                mskb = s["msk"][:, :, :, :].unsqueeze(3).broadcast_to(
                    [P, g, 2, 4, SS])
                mul_eng.tensor_tensor(e8, e8, mskb, OP.mult)
                nc.scalar.activation(e8, e8, AF.Square, scale=SQ5,
                                     accum_out=slot(q, 2))

            def class_chunk(nc, q, j, sub_eng, mul_eng):
                g = GRPS[q]
                w = CLS_CHUNKS[q][j]
                t4 = tb[q][:, :, :, :][:, :, 4, :]
                e = tcl[q][j]
                sub_eng.tensor_tensor(e, pc[q][j], e, OP.subtract)
                t4b = t4.unsqueeze(2).broadcast_to([P, g, w, SS])
                mul_eng.tensor_tensor(e, e, t4b, OP.mult)
                nc.scalar.activation(e, e, AF.Square, scale=1.0,
                                     accum_out=slot(q, 3 + j))

            V, G = nc.vector, nc.gpsimd

            # ---- emission order (per-engine queues are in-order),
            # ---- interleaved by expected readiness ----
            box_phase(nc, 0)
            box_phase(nc, 1)
            coord_phase(nc, 0, G, V)
            box_phase(nc, 2)
            class_chunk(nc, 0, 0, G, V)
            class_chunk(nc, 0, 1, G, V)
            coord_phase(nc, 1, G, V)
            class_chunk(nc, 0, 2, G, V)
            class_chunk(nc, 0, 3, G, V)
            coord_phase(nc, 2, G, V)
            class_chunk(nc, 0, 4, G, V)
            for j in range(len(CLS_CHUNKS[1])):
                sub_eng, mul_eng = (G, G) if j == 2 else (G, V)
                class_chunk(nc, 1, j, sub_eng, mul_eng)
            for j in range(len(CLS_CHUNKS[2])):
                # short tail: small trailing chunks all-DVE
                eng = (G, V) if j < 4 else (V, V)
                class_chunk(nc, 2, j, *eng)

            nc.sync.dma_start(out=out[:, :], in_=acc)
    nc.compile()
    return nc


def _get_nc():
    if "nc" not in _CACHE:
        _CACHE["nc"] = _build()
    return _CACHE["nc"]


def kernel(pred: np.ndarray, target: np.ndarray) -> np.ndarray:
    nc = _get_nc()
    in_maps = []
    for k in range(NCORE):
        sl = slice(k * NPC, (k + 1) * NPC)
        in_maps.append({
            "pred": np.ascontiguousarray(pred[sl]).reshape(NPC, D * SS),
            "target": np.ascontiguousarray(target[sl]).reshape(NPC, D * SS),
        })
    res = run_bass_kernel_spmd(nc, in_maps, core_ids=list(range(NCORE)))
    total = sum(float(r["out"].astype(np.float64).sum()) for r in res.results)
    return np.float32(total / N)
